# revision 28
# baseline (speedup 1.0000x reference)
"""AttentionPairBias — TRN2 Bass kernel, 8-core SPMD.

Sharding: data-parallel over (batch, query-row-block). Core c handles
batch c//4 and query rows [r*128, (r+1)*128) with r = c%4. Inputs are
rolled along the token axis per core so each core's own rows sit at
index 0 — every core then runs the identical program (pure SPMD).

z-path: host pre-transposes each core's z slice to [CZ, k, q] bf16.
LayerNorm over CZ is folded into the projection algebraically:
  zb = rstd*(Y - m*S) (+ Wz@lnz_b dropped: constant along softmax axis)
with Y/m/E[z^2] all produced by per-k matmuls whose stationary operand
is the [CZ, 128q] pixel tile, so outputs land directly in
[q-partition, head] layout (no on-chip transposes, lane-full epilogue).
"""

from contextlib import ExitStack

import numpy as np
import ml_dtypes

import concourse.bacc as bacc
import concourse.mybir as mybir
import concourse.tile as tile
import concourse.bass as bass
from concourse.bass import ds, ts
from concourse.bass_utils import run_bass_kernel_spmd
from concourse.masks import make_identity



B, N, C, H, D, CS, CZ = 2, 512, 768, 16, 48, 384, 128
QB = 128          # query rows per core
NCORE = 8
KC = 32           # k per zT DMA chunk
KG = 16           # k per zb psum group
EPS = 1e-5

bf16 = mybir.dt.bfloat16
f32 = mybir.dt.float32
AF = mybir.ActivationFunctionType
ALU = mybir.AluOpType
BF = ml_dtypes.bfloat16

LAST_RESULT = None  # BassKernelResults of the most recent run (for test.py)


CP = 1024  # padded channel count: 16 heads x 64 (head pitch 64, d=48 used)


def build_nc():
    nc = bacc.Bacc("TRN2", target_bir_lowering=False, debug=False,
                   num_devices=NCORE, dynamic_dma_scratch_size=8192)

    # ---- DRAM I/O (per-core shard shapes) ----
    zT_d = nc.dram_tensor("zT", [CZ, N, QB], bf16, kind="ExternalInput")
    a_d = nc.dram_tensor("a", [N, C], bf16, kind="ExternalInput")
    s_d = nc.dram_tensor("s", [N, CS], bf16, kind="ExternalInput")
    str_d = nc.dram_tensor("sTr", [CS, QB], bf16, kind="ExternalInput")
    beta_d = nc.dram_tensor("beta", [QB, N], bf16, kind="ExternalInput")
    wq_d = nc.dram_tensor("wqT", [C, CP], bf16, kind="ExternalInput")
    wk_d = nc.dram_tensor("wkT", [C, CP], bf16, kind="ExternalInput")
    wg_d = nc.dram_tensor("wgT", [C, CP], bf16, kind="ExternalInput")
    wv_d = nc.dram_tensor("wvT", [C, C], bf16, kind="ExternalInput")
    wo_d = nc.dram_tensor("woT", [CP, C], bf16, kind="ExternalInput")
    wl_d = nc.dram_tensor("wlT", [CS, C], bf16, kind="ExternalInput")
    wnb_d = nc.dram_tensor("wnbT", [CS, C], bf16, kind="ExternalInput")
    wgs_d = nc.dram_tensor("wgsT", [CS, C], bf16, kind="ExternalInput")
    w2_d = nc.dram_tensor("w2", [CZ, 17], bf16, kind="ExternalInput")
    ones_d = nc.dram_tensor("onesc", [CZ, 1], bf16, kind="ExternalInput")
    sv_d = nc.dram_tensor("negsv", [1, 16], f32, kind="ExternalInput")
    bq_d = nc.dram_tensor("bqv", [8, 128], f32, kind="ExternalInput")
    bl_d = nc.dram_tensor("blv", [6, 128], f32, kind="ExternalInput")
    bgs_d = nc.dram_tensor("bgsv", [1, C], bf16, kind="ExternalInput")
    o_d = nc.dram_tensor("o", [QB, C], f32, kind="ExternalOutput")
    zstage_d = nc.dram_tensor("zstage", [N, 32, QB], bf16)

    with tile.TileContext(nc) as tc, ExitStack() as ctx:
        consts = ctx.enter_context(tc.tile_pool(name="consts", bufs=1))
        wpool = ctx.enter_context(tc.tile_pool(name="wpool", bufs=1))
        big = ctx.enter_context(tc.tile_pool(name="big", bufs=1))
        zstream = ctx.enter_context(tc.tile_pool(name="zstream", bufs=2))
        anat = ctx.enter_context(tc.tile_pool(name="anat", bufs=2))
        smalls = ctx.enter_context(tc.tile_pool(name="smalls", bufs=3))
        zepi = ctx.enter_context(tc.tile_pool(name="zepi", bufs=2))
        sgp = ctx.enter_context(tc.tile_pool(name="sgp", bufs=2))
        expp = ctx.enter_context(tc.tile_pool(name="expp", bufs=2))
        attp = ctx.enter_context(tc.tile_pool(name="attp", bufs=2))
        mmps = ctx.enter_context(tc.tile_pool(name="mmps", bufs=3, space="PSUM"))
        zbps = ctx.enter_context(tc.tile_pool(name="zbps", bufs=2, space="PSUM"))
        stps = ctx.enter_context(tc.tile_pool(name="stps", bufs=1, space="PSUM"))
        trps = ctx.enter_context(tc.tile_pool(name="trps", bufs=2, space="PSUM"))

        # ---- constants / weights into SBUF ----
        ident = consts.tile([128, 128], bf16)
        make_identity(nc, ident[:])
        eps_sb = consts.tile([128, 1], f32)
        nc.vector.memset(eps_sb[:], EPS)
        w2_sb = consts.tile([CZ, 17], bf16)
        nc.sync.dma_start(w2_sb[:], w2_d[:])
        ones_sb = consts.tile([CZ, 1], bf16)
        nc.sync.dma_start(ones_sb[:], ones_d[:])
        nsv_sb = consts.tile([128, 16], f32)
        nc.gpsimd.dma_start(nsv_sb[:], sv_d[:].to_broadcast((128, 16)))
        bq_sb = consts.tile([128, 8], f32)
        nc.sync.dma_start(bq_sb[:], bq_d[:].rearrange("j p -> p j"))
        bl_sb = consts.tile([128, 6], f32)
        nc.sync.dma_start(bl_sb[:], bl_d[:].rearrange("j p -> p j"))
        bgs_sb = consts.tile([128, C], bf16)
        nc.gpsimd.dma_start(bgs_sb[:], bgs_d[:].to_broadcast((128, C)))
        beta_sb = consts.tile([QB, N], bf16)
        nc.sync.dma_start(beta_sb[:], beta_d[:])
        sTr_sb = consts.tile([128, 3, QB], bf16)
        nc.sync.dma_start(sTr_sb[:], str_d[:].rearrange("(j p) t -> p j t", p=128))

        wq_sb = wpool.tile([128, 6, CP], bf16)
        nc.sync.dma_start(wq_sb[:], wq_d[:].rearrange("(j p) co -> p j co", p=128))
        wk_sb = wpool.tile([128, 6, CP], bf16)
        nc.sync.dma_start(wk_sb[:], wk_d[:].rearrange("(j p) co -> p j co", p=128))
        wg_sb = wpool.tile([128, 6, CP], bf16)
        nc.sync.dma_start(wg_sb[:], wg_d[:].rearrange("(j p) co -> p j co", p=128))
        wv_sb = wpool.tile([128, 6, C], bf16)
        nc.sync.dma_start(wv_sb[:], wv_d[:].rearrange("(j p) co -> p j co", p=128))
        wo_sb = wpool.tile([128, 8, C], bf16)
        nc.sync.dma_start(wo_sb[:], wo_d[:].rearrange("(j p) co -> p j co", p=128))
        wl_sb = wpool.tile([128, 3, C], bf16)
        nc.sync.dma_start(wl_sb[:], wl_d[:].rearrange("(j p) co -> p j co", p=128))
        wnb_sb = wpool.tile([128, 3, C], bf16)
        nc.sync.dma_start(wnb_sb[:], wnb_d[:].rearrange("(j p) co -> p j co", p=128))
        wgs_sb = wpool.tile([128, 3, C], bf16)
        nc.sync.dma_start(wgs_sb[:], wgs_d[:].rearrange("(j p) co -> p j co", p=128))

        # ---- LN(a_i) -> lnaT [c, t];  LN(s_i)*w (w folded) -> lnsT ----
        lnaT = big.tile([128, 6, N], bf16)
        lnsT = big.tile([128, 3, N], bf16)
        for tt_ in range(4):
            at = anat.tile([128, C], bf16)
            nc.sync.dma_start(at[:], a_d[ts(tt_, 128), :])
            sta = smalls.tile([128, 3, 6], f32)
            for i3 in range(3):
                nc.vector.bn_stats(sta[:, i3, :], at[:, ds(i3 * 256, 256)])
            mva = smalls.tile([128, 2], f32)
            nc.vector.bn_aggr(mva[:], sta[:])
            stda = smalls.tile([128, 1], f32)
            nc.scalar.activation(stda[:], mva[:, 1:2], AF.Sqrt, bias=eps_sb[:])
            rsta = smalls.tile([128, 1], f32)
            nc.vector.reciprocal(rsta[:], stda[:])
            nc.vector.tensor_scalar(at[:], at[:], scalar1=mva[:, 0:1],
                                    scalar2=rsta[:], op0=ALU.subtract,
                                    op1=ALU.mult)
            for j in range(6):
                tp = trps.tile([128, 128], bf16, tag="tr")
                nc.tensor.transpose(tp[:], at[:, ts(j, 128)], ident[:])
                nc.scalar.copy(lnaT[:, j, ts(tt_, 128)], tp[:])

            st_ = anat.tile([128, CS], bf16)
            nc.sync.dma_start(st_[:], s_d[ts(tt_, 128), :])
            sts = smalls.tile([128, 3, 6], f32)
            for i3 in range(3):
                nc.vector.bn_stats(sts[:, i3, :], st_[:, ds(i3 * 128, 128)])
            mvs = smalls.tile([128, 2], f32)
            nc.vector.bn_aggr(mvs[:], sts[:])
            stds = smalls.tile([128, 1], f32)
            nc.scalar.activation(stds[:], mvs[:, 1:2], AF.Sqrt, bias=eps_sb[:])
            rsts = smalls.tile([128, 1], f32)
            nc.vector.reciprocal(rsts[:], stds[:])
            nc.vector.tensor_scalar(st_[:], st_[:], scalar1=mvs[:, 0:1],
                                    scalar2=rsts[:], op0=ALU.subtract,
                                    op1=ALU.mult)
            for j in range(3):
                tp = trps.tile([128, 128], bf16, tag="tr")
                nc.tensor.transpose(tp[:], st_[:, ts(j, 128)], ident[:])
                nc.scalar.copy(lnsT[:, j, ts(tt_, 128)], tp[:])

        # ---- adaLN conditioning: acondT = sigmoid(u+bl) * lnaT + v ----
        acondT = big.tile([128, 6, N], bf16)
        for j in range(6):
            u = mmps.tile([128, N], f32, tag="mm")
            for c3 in range(3):
                nc.tensor.matmul(u[:], wl_sb[:, c3, ts(j, 128)], lnsT[:, c3, :],
                                 start=(c3 == 0), stop=(c3 == 2))
            sig = sgp.tile([128, N], bf16)
            nc.scalar.activation(sig[:], u[:], AF.Sigmoid,
                                 bias=bl_sb[:, j:j + 1], scale=1.0)
            v2 = mmps.tile([128, N], f32, tag="mm")
            for c3 in range(3):
                nc.tensor.matmul(v2[:], wnb_sb[:, c3, ts(j, 128)], lnsT[:, c3, :],
                                 start=(c3 == 0), stop=(c3 == 2))
            tmpm = sgp.tile([128, N], bf16)
            nc.vector.tensor_mul(tmpm[:], sig[:], lnaT[:, j, :])
            nc.vector.tensor_add(acondT[:, j, :], tmpm[:], v2[:])

        # ---- projections: KT (transposed), V (natural), QT, GT ----
        kT = big.tile([128, 8, N], bf16)
        for j in range(8):
            kp = mmps.tile([128, N], f32, tag="mm")
            for c6 in range(6):
                nc.tensor.matmul(kp[:], wk_sb[:, c6, ts(j, 128)],
                                 acondT[:, c6, :], start=(c6 == 0),
                                 stop=(c6 == 5))
            nc.scalar.copy(kT[:, j, :], kp[:])

        v_sb = big.tile([128, 4, C], bf16)
        for tt_ in range(4):
            for hf in range(2):
                vp = mmps.tile([128, 384], f32, tag="mm")
                for c6 in range(6):
                    nc.tensor.matmul(vp[:], acondT[:, c6, ts(tt_, 128)],
                                     wv_sb[:, c6, ds(hf * 384, 384)],
                                     start=(c6 == 0), stop=(c6 == 5))
                nc.scalar.copy(v_sb[:, tt_, ds(hf * 384, 384)], vp[:])

        qT = big.tile([128, 8, QB], bf16)
        gT = big.tile([128, 8, QB], bf16)
        for j in range(8):
            qp = mmps.tile([128, QB], f32, tag="mm")
            for c6 in range(6):
                nc.tensor.matmul(qp[:], wq_sb[:, c6, ts(j, 128)],
                                 acondT[:, c6, 0:QB], start=(c6 == 0),
                                 stop=(c6 == 5))
            nc.scalar.activation(qT[:, j, :], qp[:], AF.Identity,
                                 bias=bq_sb[:, j:j + 1], scale=1.0)
            gp = mmps.tile([128, QB], f32, tag="mm")
            for c6 in range(6):
                nc.tensor.matmul(gp[:], wg_sb[:, c6, ts(j, 128)],
                                 acondT[:, c6, 0:QB], start=(c6 == 0),
                                 stop=(c6 == 5))
            nc.scalar.activation(gT[:, j, :], gp[:], AF.Sigmoid)

        # ---- z path ----
        # Stream Y = W2aug.T @ zT (pixels moving, 17-col stationary) and
        # E[z^2] (1-col ones stationary); pack 3 512-pixel blocks per PSUM
        # bank; stage to DRAM [k, 32slots, q]; one xbar DMA-transpose back
        # to [q, (k, slot)]; stats+epilogue in natural [q, k] layout.
        zbT = big.tile([QB, N, 32], bf16)
        nblk = N * QB // 512          # 128 blocks (4 k each)
        bpc = KC * QB // 512          # blocks per DMA chunk
        zts, sqs = {}, {}
        for ck in range(N // KC):
            k0 = ck * KC
            zt = zstream.tile([CZ, KC, QB], bf16)
            nc.sync.dma_start(zt[:], zT_d[:, k0:k0 + KC, :])
            sq = zstream.tile([CZ, KC, QB], bf16)
            if ck % 2 == 0:
                nc.vector.tensor_mul(sq[:], zt[:], zt[:])
            else:
                nc.gpsimd.tensor_mul(sq[:], zt[:], zt[:])
            zts[ck], sqs[ck] = zt, sq
        for t in range((nblk + 2) // 3):
            blocks = [b for b in (3 * t, 3 * t + 1, 3 * t + 2) if b < nblk]
            nb = len(blocks)
            Yp = zbps.tile([128, 512], f32)
            Ep = stps.tile([65, 512], f32)
            for i, b in enumerate(blocks):
                ck, off = divmod(b, bpc)
                rhs_z = zts[ck][:].rearrange("c k q -> c (k q)")[:, off * 512:(off + 1) * 512]
                rhs_s = sqs[ck][:].rearrange("c k q -> c (k q)")[:, off * 512:(off + 1) * 512]
                nc.tensor.matmul(Yp[32 * i:32 * i + 17, :], w2_sb[:, 0:17],
                                 rhs_z, start=True, stop=True)
                nc.tensor.matmul(Ep[32 * i:32 * i + 1, :], ones_sb[:, 0:1],
                                 rhs_s, start=True, stop=True)
            ysb = anat.tile([96, 512], bf16, tag="ysb")
            nc.scalar.copy(ysb[0:32 * nb, :], Yp[0:32 * nb, :])
            esb = anat.tile([65, 512], bf16, tag="esb")
            nc.vector.tensor_copy(esb[0:32 * (nb - 1) + 1, :],
                                  Ep[0:32 * (nb - 1) + 1, :])
            k0t = 12 * t
            for i, b in enumerate(blocks):
                nc.sync.dma_start(
                    zstage_d[4 * b:4 * b + 4, :, :]
                    .rearrange("k s q -> s k q"),
                    ysb[32 * i:32 * i + 32, :]
                    .rearrange("s (kl q) -> s kl q", kl=4))
            eap = bass.AP(tensor=esb.tensor, offset=esb[:].offset,
                          ap=[[32 * 512, nb], [128, 4], [1, 128]])
            nc.sync.dma_start(
                zstage_d[k0t:k0t + 4 * nb, 17, :]
                .rearrange("(b kl) q -> b kl q", b=nb),
                eap)
        nc.sync.dma_start_transpose(
            zbT[:], zstage_d[:].rearrange("k s q -> (k s) q"))
        # stats + epilogue in [q, k]; zb written in place over Y slots
        mV = zbT[:, :, 16]
        eV = zbT[:, :, 17]
        m2 = big.tile([QB, N], f32, tag="zm2")
        nc.vector.tensor_mul(m2[:], mV, mV)
        var = big.tile([QB, N], f32, tag="zvar")
        nc.vector.tensor_sub(var[:], eV, m2[:])
        stdv = big.tile([QB, N], f32, tag="zm2")
        nc.scalar.activation(stdv[:], var[:], AF.Sqrt, bias=eps_sb[:])
        rstd = big.tile([QB, N], f32, tag="zvar")
        nc.vector.reciprocal(rstd[:], stdv[:])
        rstdb = big.tile([QB, N, 1], bf16, tag="zrstdb")
        nc.scalar.copy(rstdb[:, :, 0], rstd[:])
        t1 = big.tile([QB, N], bf16, tag="zt1")
        nc.vector.tensor_mul(t1[:], mV, rstd[:])
        Yall = zbT[:, :, 0:16]
        nc.vector.tensor_mul(Yall, Yall, rstdb[:].to_broadcast((QB, N, 16)))
        for h in range(H):
            nc.vector.scalar_tensor_tensor(
                out=zbT[:, :, h], in0=t1[:], scalar=nsv_sb[:, h:h + 1],
                in1=zbT[:, :, h], op0=ALU.mult, op1=ALU.add)
        beta3 = beta_sb[:].rearrange("p (k o) -> p k o", o=1).to_broadcast((QB, N, 16))
        nc.vector.tensor_add(Yall, Yall, beta3)

        # ---- attention per head ----
        oT_sb = big.tile([128, 8, QB], bf16)
        nc.vector.memset(oT_sb[:], 0.0)
        for h in range(H):
            j, off = divmod(h, 2)
            off *= 64
            sc = mmps.tile([QB, N], f32, tag="mm")
            nc.tensor.matmul(sc[:], qT[off:off + D, j, :], kT[off:off + D, j, :],
                             start=True, stop=False)
            nc.tensor.matmul(sc[:], ident[:], zbT[:, :, h],
                             start=False, stop=True)
            nmax = smalls.tile([QB, 1], f32)
            nc.vector.tensor_reduce(nmax[:], sc[:], axis=mybir.AxisListType.X,
                                    op=ALU.max, negate=True)
            ex = expp.tile([QB, N], bf16)
            rs = smalls.tile([QB, 1], f32)
            nc.scalar.activation(ex[:], sc[:], AF.Exp, bias=nmax[:],
                                 scale=1.0, accum_out=rs[:])
            rc = smalls.tile([QB, 1], f32)
            nc.vector.reciprocal(rc[:], rs[:])
            nc.vector.tensor_scalar_mul(ex[:], ex[:], rc[:])
            tpm = trps.tile([128, 4, 128], bf16, tag="tr")
            for c4 in range(4):
                nc.tensor.transpose(tpm[:, c4, :], ex[:, ts(c4, 128)], ident[:])
            atT = attp.tile([128, 4, 128], bf16)
            nc.scalar.copy(atT[:], tpm[:])
            op_ = mmps.tile([D, QB], f32, tag="mm")
            for c4 in range(4):
                nc.tensor.matmul(op_[:], v_sb[:, c4, ds(h * D, D)],
                                 atT[:, c4, :], start=(c4 == 0), stop=(c4 == 3))
            nc.scalar.copy(oT_sb[off:off + D, j, :], op_[:])

        # ---- output: (g*o) @ Wo.T, final gate from raw s ----
        gated = big.tile([128, 8, QB], bf16)
        nc.vector.tensor_mul(gated[:], gT[:], oT_sb[:])
        out_sb = big.tile([QB, C], f32)
        for hf in range(2):
            gp2 = mmps.tile([QB, 384], f32, tag="mm")
            for c3 in range(3):
                nc.tensor.matmul(gp2[:], sTr_sb[:, c3, :],
                                 wgs_sb[:, c3, ds(hf * 384, 384)],
                                 start=(c3 == 0), stop=(c3 == 2))
            nc.vector.tensor_add(gp2[:], gp2[:], bgs_sb[:, ds(hf * 384, 384)])
            sgf = sgp.tile([QB, 384], bf16)
            nc.scalar.activation(sgf[:], gp2[:], AF.Sigmoid)
            op2 = mmps.tile([QB, 384], f32, tag="mm")
            for c8 in range(8):
                nc.tensor.matmul(op2[:], gated[:, c8, :],
                                 wo_sb[:, c8, ds(hf * 384, 384)],
                                 start=(c8 == 0), stop=(c8 == 7))
            nc.vector.tensor_mul(out_sb[:, ds(hf * 384, 384)], sgf[:], op2[:])
        nc.sync.dma_start(o_d[:], out_sb[:])

    nc.finalize()
    return nc


_NC = None


def _get_nc():
    global _NC
    if _NC is None:
        _NC = build_nc()
    return _NC


def _prep_maps(inputs):
    g = lambda k: np.asarray(inputs[k], np.float32)
    a_i, s_i, z_ij, beta_ij = g("a_i"), g("s_i"), g("z_ij"), g("beta_ij")
    sc = 1.0 / np.sqrt(D)

    def pad_co(w):  # [ci, 768] -> [ci, 1024], head pitch 48 -> 64
        out = np.zeros((w.shape[0], CP), np.float32)
        for h in range(H):
            out[:, h * 64:h * 64 + D] = w[:, h * D:(h + 1) * D]
        return np.ascontiguousarray(out.astype(BF))

    def pad_ci(w):  # [768, co] -> [1024, co]
        out = np.zeros((CP, w.shape[1]), np.float32)
        for h in range(H):
            out[h * 64:h * 64 + D] = w[h * D:(h + 1) * D]
        return np.ascontiguousarray(out.astype(BF))

    wqT = pad_co(g("Wq").T * sc)
    wkT = pad_co(g("Wk").T)
    wvT = np.ascontiguousarray(g("Wv").T.astype(BF))
    wgT = pad_co(g("Wg").T)
    woT = pad_ci(g("Wo").T)
    w_ln2 = g("ada_ln2_w")
    wlT = np.ascontiguousarray((g("ada_Wl") * w_ln2[None, :]).T.astype(BF))
    wnbT = np.ascontiguousarray((g("ada_Wnb") * w_ln2[None, :]).T.astype(BF))
    wgsT = np.ascontiguousarray(g("Wgs").T.astype(BF))
    bq_pad = np.zeros(CP, np.float32)
    for h in range(H):
        bq_pad[h * 64:h * 64 + D] = (g("bq") * sc)[h * D:(h + 1) * D]
    bq = bq_pad.reshape(8, 128)
    bl = g("ada_bl").astype(np.float32).reshape(6, 128)
    bgs = g("bgs").astype(BF)[None, :]
    W2 = (g("Wz") * g("lnz_w")[None, :]).T            # [CZ, H]
    negsv = np.ascontiguousarray(-W2.sum(0).astype(np.float32)[None, :])
    w2c = np.ascontiguousarray(
        np.concatenate([W2, np.full((CZ, 1), 1.0 / CZ)], 1).astype(BF))
    onesc = np.full((CZ, 1), 1.0 / CZ, BF)

    a_bf = a_i.astype(BF)
    s_bf = s_i.astype(BF)
    beta_bf = beta_ij.astype(BF)

    maps = []
    for c in range(NCORE):
        b, r = divmod(c, 4)
        q0 = r * QB
        roll = np.r_[q0:N, 0:q0]
        zc = np.ascontiguousarray(
            z_ij[b, q0:q0 + QB][:, roll, :].astype(BF).transpose(2, 1, 0))
        maps.append(dict(
            zT=zc,
            a=np.ascontiguousarray(a_bf[b][roll]),
            s=np.ascontiguousarray(s_bf[b][roll]),
            sTr=np.ascontiguousarray(s_bf[b, q0:q0 + QB].T),
            beta=np.ascontiguousarray(beta_bf[b, q0:q0 + QB][:, roll]),
            wqT=wqT, wkT=wkT, wgT=wgT, wvT=wvT, woT=woT,
            wlT=wlT, wnbT=wnbT, wgsT=wgsT,
            w2=w2c, onesc=onesc, negsv=negsv,
            bqv=bq, blv=bl, bgsv=bgs,
        ))
    return maps


def kernel(**inputs):
    global LAST_RESULT
    nc = _get_nc()
    maps = _prep_maps(inputs)
    LAST_RESULT = run_bass_kernel_spmd(nc, maps, core_ids=list(range(NCORE)))
    out = np.zeros((B, N, C), np.float32)
    for c in range(NCORE):
        b, r = divmod(c, 4)
        out[b, r * QB:(r + 1) * QB] = LAST_RESULT.results[c]["o"]
    return out


# revision 29
# speedup vs baseline: 1.3578x; 1.3578x over previous
"""AttentionPairBias — TRN2 Bass kernel, 8-core SPMD.

Sharding: data-parallel over (batch, query-row-block). Core c handles
batch c//4 and query rows [r*128, (r+1)*128) with r = c%4. Inputs are
rolled along the token axis per core so each core's own rows sit at
index 0 — every core then runs the identical program (pure SPMD).

z-path: host pre-transposes each core's z slice to [CZ, k, q] bf16.
LayerNorm over CZ is folded into the projection algebraically:
  zb = rstd*(Y - m*S) (+ Wz@lnz_b dropped: constant along softmax axis)
with Y/m/E[z^2] all produced by per-k matmuls whose stationary operand
is the [CZ, 128q] pixel tile, so outputs land directly in
[q-partition, head] layout (no on-chip transposes, lane-full epilogue).
"""

from contextlib import ExitStack

import numpy as np
import ml_dtypes

import concourse.bacc as bacc
import concourse.mybir as mybir
import concourse.tile as tile
import concourse.bass as bass
from concourse.bass import ds, ts
from concourse.bass_utils import run_bass_kernel_spmd
from concourse.masks import make_identity



B, N, C, H, D, CS, CZ = 2, 512, 768, 16, 48, 384, 128
QB = 128          # query rows per core
NCORE = 8
KC = 32           # k per zT DMA chunk
KG = 16           # k per zb psum group
EPS = 1e-5

bf16 = mybir.dt.bfloat16
f32 = mybir.dt.float32
AF = mybir.ActivationFunctionType
ALU = mybir.AluOpType
BF = ml_dtypes.bfloat16

LAST_RESULT = None  # BassKernelResults of the most recent run (for test.py)


CP = 1024  # padded channel count: 16 heads x 64 (head pitch 64, d=48 used)


def build_nc():
    nc = bacc.Bacc("TRN2", target_bir_lowering=False, debug=False,
                   num_devices=NCORE, dynamic_dma_scratch_size=8192)

    # ---- DRAM I/O (per-core shard shapes) ----
    zT_d = nc.dram_tensor("zT", [CZ, N, QB], bf16, kind="ExternalInput")
    a_d = nc.dram_tensor("a", [N, C], bf16, kind="ExternalInput")
    s_d = nc.dram_tensor("s", [N, CS], bf16, kind="ExternalInput")
    str_d = nc.dram_tensor("sTr", [CS, QB], bf16, kind="ExternalInput")
    beta_d = nc.dram_tensor("beta", [QB, N], bf16, kind="ExternalInput")
    wq_d = nc.dram_tensor("wqT", [C, CP], bf16, kind="ExternalInput")
    wk_d = nc.dram_tensor("wkT", [C, CP], bf16, kind="ExternalInput")
    wg_d = nc.dram_tensor("wgT", [C, CP], bf16, kind="ExternalInput")
    wv_d = nc.dram_tensor("wvT", [C, C], bf16, kind="ExternalInput")
    wo_d = nc.dram_tensor("woT", [CP, C], bf16, kind="ExternalInput")
    wl_d = nc.dram_tensor("wlT", [CS, C], bf16, kind="ExternalInput")
    wnb_d = nc.dram_tensor("wnbT", [CS, C], bf16, kind="ExternalInput")
    wgs_d = nc.dram_tensor("wgsT", [CS, C], bf16, kind="ExternalInput")
    w2_d = nc.dram_tensor("w2", [CZ, 17], bf16, kind="ExternalInput")
    ones_d = nc.dram_tensor("onesc", [CZ, 1], bf16, kind="ExternalInput")
    sv_d = nc.dram_tensor("negsv", [1, 16], f32, kind="ExternalInput")
    bq_d = nc.dram_tensor("bqv", [8, 128], f32, kind="ExternalInput")
    bl_d = nc.dram_tensor("blv", [6, 128], f32, kind="ExternalInput")
    bgs_d = nc.dram_tensor("bgsv", [1, C], bf16, kind="ExternalInput")
    o_d = nc.dram_tensor("o", [QB, C], f32, kind="ExternalOutput")
    zstage_d = nc.dram_tensor("zstage", [18, N, QB], bf16)

    with tile.TileContext(nc) as tc, ExitStack() as ctx:
        consts = ctx.enter_context(tc.tile_pool(name="consts", bufs=1))
        wpool = ctx.enter_context(tc.tile_pool(name="wpool", bufs=1))
        big = ctx.enter_context(tc.tile_pool(name="big", bufs=1))
        zstream = ctx.enter_context(tc.tile_pool(name="zstream", bufs=2))
        anat = ctx.enter_context(tc.tile_pool(name="anat", bufs=2))
        smalls = ctx.enter_context(tc.tile_pool(name="smalls", bufs=3))
        zepi = ctx.enter_context(tc.tile_pool(name="zepi", bufs=2))
        sgp = ctx.enter_context(tc.tile_pool(name="sgp", bufs=2))
        expp = ctx.enter_context(tc.tile_pool(name="expp", bufs=2))
        attp = ctx.enter_context(tc.tile_pool(name="attp", bufs=2))
        mmps = ctx.enter_context(tc.tile_pool(name="mmps", bufs=3, space="PSUM"))
        zbps = ctx.enter_context(tc.tile_pool(name="zbps", bufs=2, space="PSUM"))
        stps = ctx.enter_context(tc.tile_pool(name="stps", bufs=1, space="PSUM"))
        trps = ctx.enter_context(tc.tile_pool(name="trps", bufs=2, space="PSUM"))

        # ---- constants / weights into SBUF ----
        ident = consts.tile([128, 128], bf16)
        make_identity(nc, ident[:])
        eps_sb = consts.tile([128, 1], f32)
        nc.vector.memset(eps_sb[:], EPS)
        w2_sb = consts.tile([CZ, 17], bf16)
        nc.sync.dma_start(w2_sb[:], w2_d[:])
        ones_sb = consts.tile([CZ, 1], bf16)
        nc.sync.dma_start(ones_sb[:], ones_d[:])
        nsv_sb = consts.tile([128, 16], f32)
        nc.gpsimd.dma_start(nsv_sb[:], sv_d[:].to_broadcast((128, 16)))
        bq_sb = consts.tile([128, 8], f32)
        nc.sync.dma_start(bq_sb[:], bq_d[:].rearrange("j p -> p j"))
        bl_sb = consts.tile([128, 6], f32)
        nc.sync.dma_start(bl_sb[:], bl_d[:].rearrange("j p -> p j"))
        bgs_sb = consts.tile([128, C], bf16)
        nc.gpsimd.dma_start(bgs_sb[:], bgs_d[:].to_broadcast((128, C)))
        beta_sb = consts.tile([QB, N], bf16)
        nc.sync.dma_start(beta_sb[:], beta_d[:])
        sTr_sb = consts.tile([128, 3, QB], bf16)
        nc.sync.dma_start(sTr_sb[:], str_d[:].rearrange("(j p) t -> p j t", p=128))

        wq_sb = wpool.tile([128, 6, CP], bf16)
        nc.sync.dma_start(wq_sb[:], wq_d[:].rearrange("(j p) co -> p j co", p=128))
        wk_sb = wpool.tile([128, 6, CP], bf16)
        nc.sync.dma_start(wk_sb[:], wk_d[:].rearrange("(j p) co -> p j co", p=128))
        wg_sb = wpool.tile([128, 6, CP], bf16)
        nc.sync.dma_start(wg_sb[:], wg_d[:].rearrange("(j p) co -> p j co", p=128))
        wv_sb = wpool.tile([128, 6, C], bf16)
        nc.sync.dma_start(wv_sb[:], wv_d[:].rearrange("(j p) co -> p j co", p=128))
        wo_sb = wpool.tile([128, 8, C], bf16)
        nc.sync.dma_start(wo_sb[:], wo_d[:].rearrange("(j p) co -> p j co", p=128))
        wl_sb = wpool.tile([128, 3, C], bf16)
        nc.sync.dma_start(wl_sb[:], wl_d[:].rearrange("(j p) co -> p j co", p=128))
        wnb_sb = wpool.tile([128, 3, C], bf16)
        nc.sync.dma_start(wnb_sb[:], wnb_d[:].rearrange("(j p) co -> p j co", p=128))
        wgs_sb = wpool.tile([128, 3, C], bf16)
        nc.sync.dma_start(wgs_sb[:], wgs_d[:].rearrange("(j p) co -> p j co", p=128))

        # ---- LN(a_i) -> lnaT [c, t];  LN(s_i)*w (w folded) -> lnsT ----
        lnaT = big.tile([128, 6, N], bf16)
        lnsT = big.tile([128, 3, N], bf16)
        for tt_ in range(4):
            at = anat.tile([128, C], bf16)
            nc.sync.dma_start(at[:], a_d[ts(tt_, 128), :])
            sta = smalls.tile([128, 3, 6], f32)
            for i3 in range(3):
                nc.vector.bn_stats(sta[:, i3, :], at[:, ds(i3 * 256, 256)])
            mva = smalls.tile([128, 2], f32)
            nc.vector.bn_aggr(mva[:], sta[:])
            stda = smalls.tile([128, 1], f32)
            nc.scalar.activation(stda[:], mva[:, 1:2], AF.Sqrt, bias=eps_sb[:])
            rsta = smalls.tile([128, 1], f32)
            nc.vector.reciprocal(rsta[:], stda[:])
            nc.vector.tensor_scalar(at[:], at[:], scalar1=mva[:, 0:1],
                                    scalar2=rsta[:], op0=ALU.subtract,
                                    op1=ALU.mult)
            for j in range(6):
                tp = trps.tile([128, 128], bf16, tag="tr")
                nc.tensor.transpose(tp[:], at[:, ts(j, 128)], ident[:])
                nc.scalar.copy(lnaT[:, j, ts(tt_, 128)], tp[:])

            st_ = anat.tile([128, CS], bf16)
            nc.sync.dma_start(st_[:], s_d[ts(tt_, 128), :])
            sts = smalls.tile([128, 3, 6], f32)
            for i3 in range(3):
                nc.vector.bn_stats(sts[:, i3, :], st_[:, ds(i3 * 128, 128)])
            mvs = smalls.tile([128, 2], f32)
            nc.vector.bn_aggr(mvs[:], sts[:])
            stds = smalls.tile([128, 1], f32)
            nc.scalar.activation(stds[:], mvs[:, 1:2], AF.Sqrt, bias=eps_sb[:])
            rsts = smalls.tile([128, 1], f32)
            nc.vector.reciprocal(rsts[:], stds[:])
            nc.vector.tensor_scalar(st_[:], st_[:], scalar1=mvs[:, 0:1],
                                    scalar2=rsts[:], op0=ALU.subtract,
                                    op1=ALU.mult)
            for j in range(3):
                tp = trps.tile([128, 128], bf16, tag="tr")
                nc.tensor.transpose(tp[:], st_[:, ts(j, 128)], ident[:])
                nc.scalar.copy(lnsT[:, j, ts(tt_, 128)], tp[:])

        # ---- adaLN conditioning: acondT = sigmoid(u+bl) * lnaT + v ----
        acondT = big.tile([128, 6, N], bf16)
        for j in range(6):
            u = mmps.tile([128, N], f32, tag="mm")
            for c3 in range(3):
                nc.tensor.matmul(u[:], wl_sb[:, c3, ts(j, 128)], lnsT[:, c3, :],
                                 start=(c3 == 0), stop=(c3 == 2))
            sig = sgp.tile([128, N], bf16)
            nc.scalar.activation(sig[:], u[:], AF.Sigmoid,
                                 bias=bl_sb[:, j:j + 1], scale=1.0)
            v2 = mmps.tile([128, N], f32, tag="mm")
            for c3 in range(3):
                nc.tensor.matmul(v2[:], wnb_sb[:, c3, ts(j, 128)], lnsT[:, c3, :],
                                 start=(c3 == 0), stop=(c3 == 2))
            tmpm = sgp.tile([128, N], bf16)
            nc.vector.tensor_mul(tmpm[:], sig[:], lnaT[:, j, :])
            nc.vector.tensor_add(acondT[:, j, :], tmpm[:], v2[:])

        # ---- projections: KT (transposed), V (natural), QT, GT ----
        kT = big.tile([128, 8, N], bf16)
        for j in range(8):
            kp = mmps.tile([128, N], f32, tag="mm")
            for c6 in range(6):
                nc.tensor.matmul(kp[:], wk_sb[:, c6, ts(j, 128)],
                                 acondT[:, c6, :], start=(c6 == 0),
                                 stop=(c6 == 5))
            nc.scalar.copy(kT[:, j, :], kp[:])

        v_sb = big.tile([128, 4, C], bf16)
        for tt_ in range(4):
            for hf in range(2):
                vp = mmps.tile([128, 384], f32, tag="mm")
                for c6 in range(6):
                    nc.tensor.matmul(vp[:], acondT[:, c6, ts(tt_, 128)],
                                     wv_sb[:, c6, ds(hf * 384, 384)],
                                     start=(c6 == 0), stop=(c6 == 5))
                nc.scalar.copy(v_sb[:, tt_, ds(hf * 384, 384)], vp[:])

        qT = big.tile([128, 8, QB], bf16)
        gT = big.tile([128, 8, QB], bf16)
        for j in range(8):
            qp = mmps.tile([128, QB], f32, tag="mm")
            for c6 in range(6):
                nc.tensor.matmul(qp[:], wq_sb[:, c6, ts(j, 128)],
                                 acondT[:, c6, 0:QB], start=(c6 == 0),
                                 stop=(c6 == 5))
            nc.scalar.activation(qT[:, j, :], qp[:], AF.Identity,
                                 bias=bq_sb[:, j:j + 1], scale=1.0)
            gp = mmps.tile([128, QB], f32, tag="mm")
            for c6 in range(6):
                nc.tensor.matmul(gp[:], wg_sb[:, c6, ts(j, 128)],
                                 acondT[:, c6, 0:QB], start=(c6 == 0),
                                 stop=(c6 == 5))
            nc.scalar.activation(gT[:, j, :], gp[:], AF.Sigmoid)

        # ---- z path ----
        # Stream Y = W2aug.T @ zT (pixels moving, 17-col stationary) and
        # E[z^2] (1-col ones); pack 3 blocks per PSUM bank; stage to DRAM
        # [18 slots, k, q]; xbar DMA-transpose back in slot-pieces so
        # stats/epilogue/attention overlap; epilogue packed in [q, s, k].
        zbT = big.tile([QB, 18, N], bf16)
        nblk = N * QB // 512
        bpc = KC * QB // 512
        zts, sqs = {}, {}
        for ck in range(N // KC):
            k0 = ck * KC
            zt = zstream.tile([CZ, KC, QB], bf16)
            nc.sync.dma_start(zt[:], zT_d[:, k0:k0 + KC, :])
            sq = zstream.tile([CZ, KC, QB], bf16)
            if ck % 2 == 0:
                nc.vector.tensor_mul(sq[:], zt[:], zt[:])
            else:
                nc.gpsimd.tensor_mul(sq[:], zt[:], zt[:])
            zts[ck], sqs[ck] = zt, sq
        for t in range((nblk + 2) // 3):
            blocks = [b for b in (3 * t, 3 * t + 1, 3 * t + 2) if b < nblk]
            nb = len(blocks)
            Yp = zbps.tile([128, 512], f32, tag="Yp")
            Ep = stps.tile([65, 512], f32, tag="Ep")
            for i, b in enumerate(blocks):
                ck, off = divmod(b, bpc)
                rhs_z = zts[ck][:].rearrange("c k q -> c (k q)")[:, off * 512:(off + 1) * 512]
                rhs_s = sqs[ck][:].rearrange("c k q -> c (k q)")[:, off * 512:(off + 1) * 512]
                nc.tensor.matmul(Yp[32 * i:32 * i + 17, :], w2_sb[:, 0:17],
                                 rhs_z, start=True, stop=True)
                nc.tensor.matmul(Ep[32 * i:32 * i + 1, :], ones_sb[:, 0:1],
                                 rhs_s, start=True, stop=True)
            ysb = anat.tile([96, 512], bf16, tag="ysb")
            nc.scalar.copy(ysb[0:32 * nb, :], Yp[0:32 * nb, :])
            esb = anat.tile([65, 512], bf16, tag="esb")
            nc.vector.tensor_copy(esb[0:32 * (nb - 1) + 1, :],
                                  Ep[0:32 * (nb - 1) + 1, :])
            for i, b in enumerate(blocks):
                eng = nc.sync if (t + i) % 2 == 0 else nc.scalar
                eng.dma_start(
                    zstage_d[0:17, 4 * b:4 * b + 4, :],
                    ysb[32 * i:32 * i + 17, :]
                    .rearrange("s (kl q) -> s kl q", kl=4))
            k0t = 12 * t
            eap = bass.AP(tensor=esb.tensor, offset=esb[:].offset,
                          ap=[[32 * 512, nb], [128, 4], [1, 128]])
            nc.gpsimd.dma_start(
                zstage_d[17, k0t:k0t + 4 * nb, :]
                .rearrange("(b kl) q -> b kl q", b=nb),
                eap)
        # transpose back in slot pieces: stats first, then heads 0-7, 8-15
        nc.sync.dma_start_transpose(
            zbT[:, 16:18, :], zstage_d[16:18].rearrange("s k q -> (s k) q"))
        mV = zbT[:, 16, :]
        eV = zbT[:, 17, :]
        m2 = big.tile([QB, N], f32, tag="zm2")
        nc.vector.tensor_mul(m2[:], mV, mV)
        var = big.tile([QB, N], f32, tag="zvar")
        nc.vector.tensor_sub(var[:], eV, m2[:])
        stdv = big.tile([QB, N], f32, tag="zm2")
        nc.scalar.activation(stdv[:], var[:], AF.Sqrt, bias=eps_sb[:])
        rstd = big.tile([QB, N], f32, tag="zvar")
        nc.vector.reciprocal(rstd[:], stdv[:])
        rstdb = big.tile([QB, 1, N], bf16, tag="zrstdb")
        nc.scalar.copy(rstdb[:, 0, :], rstd[:])
        t1 = big.tile([QB, N], bf16, tag="zt1")
        nc.vector.tensor_mul(t1[:], mV, rstd[:])
        beta1 = beta_sb[:].rearrange("p (o k) -> p o k", o=1)
        for half in range(2):
            s0 = 8 * half
            nc.sync.dma_start_transpose(
                zbT[:, s0:s0 + 8, :],
                zstage_d[s0:s0 + 8].rearrange("s k q -> (s k) q"))
            Yh = zbT[:, s0:s0 + 8, :]
            nc.vector.tensor_mul(Yh, Yh, rstdb[:].to_broadcast((QB, 8, N)))
            for h in range(s0, s0 + 8):
                nc.vector.scalar_tensor_tensor(
                    out=zbT[:, h, :], in0=t1[:], scalar=nsv_sb[:, h:h + 1],
                    in1=zbT[:, h, :], op0=ALU.mult, op1=ALU.add)
            nc.vector.tensor_add(Yh, Yh, beta1.to_broadcast((QB, 8, N)))

        # ---- attention per head ----
        oT_sb = big.tile([128, 8, QB], bf16)
        nc.vector.memset(oT_sb[:], 0.0)
        for h in range(H):
            j, off = divmod(h, 2)
            off *= 64
            sc = zbps.tile([QB, N], f32, tag="Yp")
            nc.tensor.matmul(sc[:], qT[off:off + D, j, :], kT[off:off + D, j, :],
                             start=True, stop=False)
            nc.tensor.matmul(sc[:], ident[:], zbT[:, h, :],
                             start=False, stop=True)
            nmax = smalls.tile([QB, 1], f32)
            nc.vector.tensor_reduce(nmax[:], sc[:], axis=mybir.AxisListType.X,
                                    op=ALU.max, negate=True)
            ex = expp.tile([QB, N], bf16)
            rs = smalls.tile([QB, 1], f32)
            nc.scalar.activation(ex[:], sc[:], AF.Exp, bias=nmax[:],
                                 scale=1.0, accum_out=rs[:])
            rc = smalls.tile([QB, 1], f32)
            nc.vector.reciprocal(rc[:], rs[:])
            nc.vector.tensor_scalar_mul(ex[:], ex[:], rc[:])
            tpm = trps.tile([128, 4, 128], bf16, tag="tr")
            for c4 in range(4):
                nc.tensor.transpose(tpm[:, c4, :], ex[:, ts(c4, 128)], ident[:])
            atT = attp.tile([128, 4, 128], bf16)
            nc.scalar.copy(atT[:], tpm[:])
            op_ = stps.tile([D, QB], f32, tag="Ep")
            for c4 in range(4):
                nc.tensor.matmul(op_[:], v_sb[:, c4, ds(h * D, D)],
                                 atT[:, c4, :], start=(c4 == 0), stop=(c4 == 3))
            nc.scalar.copy(oT_sb[off:off + D, j, :], op_[:])

        # ---- output: (g*o) @ Wo.T, final gate from raw s ----
        gated = big.tile([128, 8, QB], bf16)
        nc.vector.tensor_mul(gated[:], gT[:], oT_sb[:])
        out_sb = big.tile([QB, C], f32)
        for hf in range(2):
            gp2 = mmps.tile([QB, 384], f32, tag="mm")
            for c3 in range(3):
                nc.tensor.matmul(gp2[:], sTr_sb[:, c3, :],
                                 wgs_sb[:, c3, ds(hf * 384, 384)],
                                 start=(c3 == 0), stop=(c3 == 2))
            nc.vector.tensor_add(gp2[:], gp2[:], bgs_sb[:, ds(hf * 384, 384)])
            sgf = sgp.tile([QB, 384], bf16)
            nc.scalar.activation(sgf[:], gp2[:], AF.Sigmoid)
            op2 = mmps.tile([QB, 384], f32, tag="mm")
            for c8 in range(8):
                nc.tensor.matmul(op2[:], gated[:, c8, :],
                                 wo_sb[:, c8, ds(hf * 384, 384)],
                                 start=(c8 == 0), stop=(c8 == 7))
            nc.vector.tensor_mul(out_sb[:, ds(hf * 384, 384)], sgf[:], op2[:])
        nc.sync.dma_start(o_d[:], out_sb[:])

    nc.finalize()
    return nc


_NC = None


def _get_nc():
    global _NC
    if _NC is None:
        _NC = build_nc()
    return _NC


def _prep_maps(inputs):
    g = lambda k: np.asarray(inputs[k], np.float32)
    a_i, s_i, z_ij, beta_ij = g("a_i"), g("s_i"), g("z_ij"), g("beta_ij")
    sc = 1.0 / np.sqrt(D)

    def pad_co(w):  # [ci, 768] -> [ci, 1024], head pitch 48 -> 64
        out = np.zeros((w.shape[0], CP), np.float32)
        for h in range(H):
            out[:, h * 64:h * 64 + D] = w[:, h * D:(h + 1) * D]
        return np.ascontiguousarray(out.astype(BF))

    def pad_ci(w):  # [768, co] -> [1024, co]
        out = np.zeros((CP, w.shape[1]), np.float32)
        for h in range(H):
            out[h * 64:h * 64 + D] = w[h * D:(h + 1) * D]
        return np.ascontiguousarray(out.astype(BF))

    wqT = pad_co(g("Wq").T * sc)
    wkT = pad_co(g("Wk").T)
    wvT = np.ascontiguousarray(g("Wv").T.astype(BF))
    wgT = pad_co(g("Wg").T)
    woT = pad_ci(g("Wo").T)
    w_ln2 = g("ada_ln2_w")
    wlT = np.ascontiguousarray((g("ada_Wl") * w_ln2[None, :]).T.astype(BF))
    wnbT = np.ascontiguousarray((g("ada_Wnb") * w_ln2[None, :]).T.astype(BF))
    wgsT = np.ascontiguousarray(g("Wgs").T.astype(BF))
    bq_pad = np.zeros(CP, np.float32)
    for h in range(H):
        bq_pad[h * 64:h * 64 + D] = (g("bq") * sc)[h * D:(h + 1) * D]
    bq = bq_pad.reshape(8, 128)
    bl = g("ada_bl").astype(np.float32).reshape(6, 128)
    bgs = g("bgs").astype(BF)[None, :]
    W2 = (g("Wz") * g("lnz_w")[None, :]).T            # [CZ, H]
    negsv = np.ascontiguousarray(-W2.sum(0).astype(np.float32)[None, :])
    w2c = np.ascontiguousarray(
        np.concatenate([W2, np.full((CZ, 1), 1.0 / CZ)], 1).astype(BF))
    onesc = np.full((CZ, 1), 1.0 / CZ, BF)

    a_bf = a_i.astype(BF)
    s_bf = s_i.astype(BF)
    beta_bf = beta_ij.astype(BF)

    maps = []
    for c in range(NCORE):
        b, r = divmod(c, 4)
        q0 = r * QB
        roll = np.r_[q0:N, 0:q0]
        zc = np.ascontiguousarray(
            z_ij[b, q0:q0 + QB][:, roll, :].astype(BF).transpose(2, 1, 0))
        maps.append(dict(
            zT=zc,
            a=np.ascontiguousarray(a_bf[b][roll]),
            s=np.ascontiguousarray(s_bf[b][roll]),
            sTr=np.ascontiguousarray(s_bf[b, q0:q0 + QB].T),
            beta=np.ascontiguousarray(beta_bf[b, q0:q0 + QB][:, roll]),
            wqT=wqT, wkT=wkT, wgT=wgT, wvT=wvT, woT=woT,
            wlT=wlT, wnbT=wnbT, wgsT=wgsT,
            w2=w2c, onesc=onesc, negsv=negsv,
            bqv=bq, blv=bl, bgsv=bgs,
        ))
    return maps


def kernel(**inputs):
    global LAST_RESULT
    nc = _get_nc()
    maps = _prep_maps(inputs)
    LAST_RESULT = run_bass_kernel_spmd(nc, maps, core_ids=list(range(NCORE)))
    out = np.zeros((B, N, C), np.float32)
    for c in range(NCORE):
        b, r = divmod(c, 4)
        out[b, r * QB:(r + 1) * QB] = LAST_RESULT.results[c]["o"]
    return out
